# revision 48
# baseline (speedup 1.0000x reference)
"""GQA kernel for trn2, 8 NeuronCores, tensor-parallel over heads.

Sharding: 4 q heads + 1 kv head per core (column-split Wq/Wk/Wv, row-split Wo),
partial outputs summed on host. All matmul operands bf16, fp32 PSUM accumulation.

v2 layout/schedule (vs baseline):
  - weights DMA'd before x; kv/q projections accumulate per arriving x chunk
    (halved over SEQ so 3 group-halves fit the PSUM budget)
  - scores row-tiled on the PE: head pair (2g, 2g+1) at tile_position (0,0)/
    (64,0) run concurrently (K=64 each) into one [128, 1024] psum tile
  - causal mask folded into scores via maskT.T @ I accumulation on diagonal
    128-chunks (no DVE tri-mul)
  - one exp per (pair, qblock, kchunk) with per-partition k-RMS scale folded;
    ACT stays in the exp table set during attention
  - softmax denominator via ones-row 64 of the PV lhsT; 1/den via DVE
    reciprocal_approx_fast; column-broadcast via gpsimd; normalize on DVE
  - Wo of block cb and second-half projections emitted as PE filler inside
    the next block's exp-wait gaps
"""

import sys
import types
import numpy as np
import ml_dtypes

for _p in ("/opt/trn_rl_repo",):
    if _p not in sys.path:
        sys.path.append(_p)

SEQ = 2048
DIM = 2048
HD = 64
NCORES = 8
EPS = 1e-6
THETA = 10000.0
QB = 512          # attention query block (1 psum bank per head)
NQB = SEQ // QB   # 4
HB = 1024         # projection column half
MASKV = -30000.0
ROWTILE = True    # run the head pair's score matmuls on PE row strips 0/1

_CACHE = {}


def _ensure_ntff_hook():
    """Re-register the NTFF profile hook the boot drops (stub antenv)."""
    if "antenv.axon_hooks" in sys.modules:
        return
    try:
        import antenv
        m = types.ModuleType("antenv.axon_hooks")
        hook = [None]
        m.set_axon_ntff_profile_hook = lambda h: hook.__setitem__(0, h)
        m.get_axon_ntff_profile_hook = lambda: hook[0]
        sys.modules["antenv.axon_hooks"] = m
        antenv.axon_hooks = m
        from trn_agent_boot.trn_boot import _ntff_profile_via_ctypes
        m.set_axon_ntff_profile_hook(
            _ntff_profile_via_ctypes("/opt/axon/libaxon_pjrt.so"))
    except Exception:
        pass


def _build_nc():
    import concourse.mybir as mybir
    import concourse.tile as tile
    from concourse import bacc

    f32, f16 = mybir.dt.float32, mybir.dt.bfloat16
    EXP = mybir.ActivationFunctionType.Exp
    LN = mybir.ActivationFunctionType.Ln
    SQR = mybir.ActivationFunctionType.Square
    CPY = mybir.ActivationFunctionType.Copy

    nc = bacc.Bacc("TRN2", target_bir_lowering=False, debug=False,
                   num_devices=NCORES)

    d_xt = nc.dram_tensor("xt", (128, 16, SEQ), f16, kind="ExternalInput")
    d_wq = nc.dram_tensor("wq", (128, 16, 256), f16, kind="ExternalInput")
    d_wkv = nc.dram_tensor("wkv", (128, 16, 128), f16, kind="ExternalInput")
    d_wo = nc.dram_tensor("wo", (128, 2, DIM), f16, kind="ExternalInput")
    d_cosq = nc.dram_tensor("cosq", (128, SEQ), f16, kind="ExternalInput")
    d_sinq = nc.dram_tensor("sinq", (128, SEQ), f16, kind="ExternalInput")
    d_cosk = nc.dram_tensor("cosk", (64, SEQ), f16, kind="ExternalInput")
    d_sink = nc.dram_tensor("sink", (64, SEQ), f16, kind="ExternalInput")
    d_tri = nc.dram_tensor("tri", (128, 128), f16, kind="ExternalInput")
    d_idn = nc.dram_tensor("idn", (128, 128), f16, kind="ExternalInput")
    d_ob = nc.dram_tensor("ob", (128, 65), f16, kind="ExternalInput")
    d_y = nc.dram_tensor("y", (16, 128, DIM), f16, kind="ExternalOutput")

    with tile.TileContext(nc) as tc:
        from contextlib import ExitStack
        with ExitStack() as ctx:
            kconst = ctx.enter_context(tc.tile_pool(name="kconst", bufs=1))
            xpool = ctx.enter_context(tc.tile_pool(name="xp", bufs=1))
            work = ctx.enter_context(tc.tile_pool(name="work", bufs=1))
            vpool = ctx.enter_context(tc.tile_pool(name="vp", bufs=1))
            epool = ctx.enter_context(tc.tile_pool(name="ep", bufs=4))
            yspool = ctx.enter_context(tc.tile_pool(name="ysp", bufs=2))
            bigps = ctx.enter_context(
                tc.tile_pool(name="bigps", bufs=2, space="PSUM"))
            smallps = ctx.enter_context(
                tc.tile_pool(name="smallps", bufs=2, space="PSUM"))

            # ---- DMAs: proj weights, then x (chunk groups), then consts
            wq_sb = kconst.tile([128, 16, 256], f16, tag="wq")
            nc.sync.dma_start(out=wq_sb, in_=d_wq[:, :, :])
            wkv_sb = kconst.tile([128, 16, 128], f16, tag="wkv")
            nc.sync.dma_start(out=wkv_sb, in_=d_wkv[:, :, :])
            XG = [(0, 2), (2, 4), (4, 8), (8, 12), (12, 16)]
            xtg = []
            for gi, (a, b) in enumerate(XG):
                t = xpool.tile([128, b - a, SEQ], f16, tag=f"xtg{gi}",
                               name=f"xtg{gi}")
                nc.sync.dma_start(out=t, in_=d_xt[:, a:b, :])
                xtg.append(t)

            def xts(k):
                for gi, (a, b) in enumerate(XG):
                    if a <= k < b:
                        return xtg[gi][:, k - a, :]

            wo_sb = kconst.tile([128, 2, DIM], f16, tag="wo")
            nc.sync.dma_start(out=wo_sb, in_=d_wo[:, :, :])
            cosq_sb = kconst.tile([128, SEQ], f16, tag="cosq")
            nc.sync.dma_start(out=cosq_sb, in_=d_cosq[:, :])
            sinq_sb = kconst.tile([128, SEQ], f16, tag="sinq")
            nc.sync.dma_start(out=sinq_sb, in_=d_sinq[:, :])
            cosk_sb = kconst.tile([64, SEQ], f16, tag="cosk")
            nc.sync.dma_start(out=cosk_sb, in_=d_cosk[:, :])
            sink_sb = kconst.tile([64, SEQ], f16, tag="sink")
            nc.sync.dma_start(out=sink_sb, in_=d_sink[:, :])
            tri_sb = kconst.tile([128, 128], f16, tag="tri")
            nc.sync.dma_start(out=tri_sb, in_=d_tri[:, :])
            idn_sb = kconst.tile([128, 128], f16, tag="idn")
            nc.sync.dma_start(out=idn_sb, in_=d_idn[:, :])
            ob_sb = kconst.tile([128, 65], f16, tag="ob")
            nc.sync.dma_start(out=ob_sb, in_=d_ob[:, :])
            eps_sb = kconst.tile([128, 1], f32, tag="eps")
            nc.vector.memset(eps_sb, EPS)
            eps64_sb = kconst.tile([128, 1], f32, tag="eps64")
            nc.vector.memset(eps64_sb, float(HD) * EPS)

            # persistent projection-phase results
            if ROWTILE:
                qro = [kconst.tile([128, SEQ], f16, tag=f"qro{g}",
                                   name=f"qro{g}") for g in range(2)]
            else:
                qro4 = [kconst.tile([64, SEQ], f16, tag=f"qro4_{i}",
                                    name=f"qro4_{i}") for i in range(4)]
            kro2 = kconst.tile([128, SEQ], f16, tag="kro2")  # kro duplicated
            rk2 = kconst.tile([128, 16], f32, tag="rk2")     # exp scale/kchunk
            rsq_ = [[kconst.tile([1, SEQ], f16, tag=f"rs{g}{r}",
                                 name=f"rs{g}{r}") for r in range(2)]
                    for g in range(2)]           # q-norm rsqrt rows
            vr = [vpool.tile([128, 65], f16, tag=f"vr{j}", name=f"vr{j}")
                  for j in range(16)]            # [v.T | 1] per 128-k-chunk
            aot = [kconst.tile([128, SEQ], f16, tag=f"aot{g}", name=f"aot{g}")
                   for g in range(2)]            # normalized attn out (Wo lhsT)

            # ---- projection helpers (per column half) --------------------
            def qpost_half(g, half, pj):
                """rope + q-norm for head pair g, seq cols half*HB..+HB."""
                h0 = HB * half
                q16 = work.tile([128, HB], f16, tag="q16", bufs=3)
                nc.scalar.activation(out=q16, in_=pj, func=CPY,
                                     scale=1.0, bias=0.0)
                sq = work.tile([128, HB], f16, tag="sq", bufs=3)
                nc.scalar.activation(out=sq, in_=pj, func=SQR,
                                     scale=1.0, bias=0.0)
                for n in range(2):
                    sp = smallps.tile([65, 512], f32, tag="small")
                    nc.tensor.matmul(sp, ob_sb,
                                     sq[:, 512 * n:512 * n + 512],
                                     start=True, stop=True)
                    for r in range(2):
                        # rsqrt(x/64 + eps) = exp(-0.5 ln(x/64 + eps)); ln
                        # and exp share one ACT table set, rsqrt does not
                        tl = work.tile([1, 512], f32, tag="tln", bufs=2)
                        nc.scalar.activation(
                            out=tl, in_=sp[64 * r:64 * r + 1, :], func=LN,
                            scale=1.0 / HD, bias=eps_sb[0:1, :])
                        nc.scalar.activation(
                            out=rsq_[g][r][:, h0 + 512 * n:h0 + 512 * n + 512],
                            in_=tl, func=EXP, scale=-0.5)
                rot = work.tile([128, HB], f16, tag="rot", bufs=2)
                for (o, s) in ((0, 32), (32, 0), (64, 96), (96, 64)):
                    nc.vector.tensor_copy(out=rot[o:o + 32, :],
                                          in_=q16[s:s + 32, :])
                for r in range(2):
                    p = 64 * r
                    bq = work.tile([64, HB], f16, tag="bq", bufs=2)
                    nc.gpsimd.partition_broadcast(
                        bq, rsq_[g][r][:, h0:h0 + HB], channels=64)
                    tm = work.tile([64, HB], f16, tag="tm", bufs=2)
                    nc.vector.tensor_mul(tm, rot[p:p + 64, :],
                                         sinq_sb[p:p + 64, h0:h0 + HB])
                    hh = work.tile([64, HB], f16, tag="hh", bufs=2)
                    nc.vector.tensor_mul(hh, q16[p:p + 64, :],
                                         cosq_sb[p:p + 64, h0:h0 + HB])
                    nc.vector.tensor_add(hh, hh, tm)
                    qdst = (qro[g][p:p + 64, h0:h0 + HB] if ROWTILE
                            else qro4[2 * g + r][:, h0:h0 + HB])
                    nc.vector.tensor_mul(qdst, hh, bq)

            def kvpost_half(half, pj):
                h0 = HB * half
                k16 = work.tile([64, HB], f16, tag="q16", bufs=3)
                nc.scalar.activation(out=k16, in_=pj[0:64, :], func=CPY,
                                     scale=1.0, bias=0.0)
                sqk = work.tile([64, HB], f16, tag="sq", bufs=3)
                nc.scalar.activation(out=sqk, in_=pj[0:64, :], func=SQR,
                                     scale=1.0, bias=0.0)
                v16 = work.tile([64, HB], f16, tag="v16", bufs=2)
                nc.scalar.activation(out=v16, in_=pj[64:128, :], func=CPY,
                                     scale=1.0, bias=0.0)
                pc = smallps.tile([128, 8], f32, tag="small")
                for j in range(8):
                    nc.tensor.matmul(pc[:, j:j + 1],
                                     sqk[:, 128 * j:128 * j + 128],
                                     ob_sb[0:64, 0:1],
                                     start=True, stop=True)
                tk = work.tile([128, 8], f32, tag="tk", bufs=2)
                nc.scalar.activation(out=tk, in_=pc, func=LN,
                                     scale=1.0, bias=eps64_sb)
                nc.scalar.activation(out=rk2[:, 8 * half:8 * half + 8],
                                     in_=tk, func=EXP, scale=-0.5)
                rotk = work.tile([64, HB], f16, tag="rot", bufs=2)
                for (o, s) in ((0, 32), (32, 0)):
                    nc.vector.tensor_copy(out=rotk[o:o + 32, :],
                                          in_=k16[s:s + 32, :])
                tmk = work.tile([64, HB], f16, tag="tm", bufs=2)
                nc.vector.tensor_mul(tmk, rotk, sink_sb[:, h0:h0 + HB])
                hk = work.tile([64, HB], f16, tag="hh", bufs=2)
                nc.vector.tensor_mul(hk, k16, cosk_sb[:, h0:h0 + HB])
                nc.vector.tensor_add(kro2[0:64, h0:h0 + HB], hk, tmk)
                nc.vector.tensor_copy(out=kro2[64:128, h0:h0 + HB],
                                      in_=kro2[0:64, h0:h0 + HB])
                for j in range(8):
                    tp = smallps.tile([128, 64], f16, tag="small")
                    nc.tensor.transpose(tp, v16[:, 128 * j:128 * j + 128],
                                        idn_sb[0:64, 0:64])
                    jj = 8 * half + j
                    nc.vector.tensor_copy(out=vr[jj][:, 0:64], in_=tp)
                    nc.vector.memset(vr[jj][:, 64:65], 1.0)

            # ---- first halves: interleaved per x chunk (DMA-paced) -------
            pj_kv0 = bigps.tile([128, HB], f32, tag="big", name="pjkv0")
            pj_g00 = bigps.tile([128, HB], f32, tag="big", name="pjg00")
            pj_g10 = bigps.tile([128, HB], f32, tag="pjh", bufs=1,
                                name="pjg10")
            for k in range(16):
                for pj, wsl in ((pj_kv0, wkv_sb[:, k, :]),
                                (pj_g00, wq_sb[:, k, 0:128]),
                                (pj_g10, wq_sb[:, k, 128:256])):
                    for n in range(2):
                        mm = nc.tensor.matmul(
                            pj[:, 512 * n:512 * n + 512], wsl,
                            xts(k)[:, 512 * n:512 * n + 512],
                            start=(k == 0), stop=(k == 15))
                        if n == 1:
                            mm.ins.ldweights = False
            kvpost_half(0, pj_kv0)
            qpost_half(0, 0, pj_g00)
            qpost_half(1, 0, pj_g10)

            # ---- second halves queued as PE filler work ------------------
            fillers = []
            state = {}

            def mk_proj_unit(key, wsl, k, name):
                def unit():
                    if k == 0:
                        state[key] = bigps.tile([128, HB], f32, tag="pjh",
                                                bufs=1, name=name)
                    pj = state[key]
                    for n in range(2):
                        c = HB + 512 * n
                        mm = nc.tensor.matmul(pj[:, 512 * n:512 * n + 512],
                                              wsl(k), xts(k)[:, c:c + 512],
                                              start=(k == 0), stop=(k == 15))
                        if n == 1:
                            mm.ins.ldweights = False
                return unit

            for k in range(16):
                fillers.append(mk_proj_unit(
                    "kv1", lambda k: wkv_sb[:, k, :], k, "pjkv1"))
            for k in range(16):
                fillers.append(mk_proj_unit(
                    "g01", lambda k: wq_sb[:, k, 0:128], k, "pjg01"))
            for k in range(16):
                fillers.append(mk_proj_unit(
                    "g11", lambda k: wq_sb[:, k, 128:256], k, "pjg11"))

            def drain(n):
                for _ in range(min(n, len(fillers))):
                    fillers.pop(0)()

            def mk_wo_unit(cb, m, hdp):
                def unit():
                    if hdp == 0:
                        state[f"ys{m}"] = yspool.tile([128, DIM], f16,
                                                      tag="ysm",
                                                      name=f"ysm{m}")
                    ysm = state[f"ys{m}"]
                    c0, c1 = 1024 * hdp, 1024 * hdp + 512
                    if cb == NQB - 1:   # tail: double-buffer via the sc slots
                        yp = bigps.tile([128, 1024], f32, tag="big",
                                        name="yp")
                    else:
                        yp = bigps.tile([128, 1024], f32, tag="pjh", bufs=1,
                                        name="yp")
                    for g in range(2):
                        nc.tensor.matmul(yp[:, 0:512],
                                         aot[g][:, 128 * m:128 * m + 128],
                                         wo_sb[:, g, c0:c0 + 512],
                                         start=(g == 0), stop=(g == 1))
                        mm = nc.tensor.matmul(
                            yp[:, 512:1024], aot[g][:, 128 * m:128 * m + 128],
                            wo_sb[:, g, c1:c1 + 512],
                            start=(g == 0), stop=(g == 1))
                        mm.ins.ldweights = False
                    if cb == NQB - 1:
                        nc.scalar.activation(out=ysm[:, c0:c0 + 512],
                                             in_=yp[:, 0:512],
                                             func=CPY, scale=1.0, bias=0.0)
                    else:
                        nc.vector.tensor_copy(out=ysm[:, c0:c0 + 512],
                                              in_=yp[:, 0:512])
                    nc.vector.tensor_copy(out=ysm[:, c1:c1 + 512],
                                          in_=yp[:, 512:1024])
                    if hdp == 1:
                        nc.sync.dma_start(out=d_y[m], in_=ysm)
                return unit

            # posts for the second halves, emitted at (cb, g) boundaries
            # where the small-psum ot slots are free (deadlock avoidance)
            boundary_posts = {
                (0, 0): lambda: kvpost_half(1, state["kv1"]),
                (0, 1): lambda: qpost_half(0, 1, state["g01"]),
                (1, 0): lambda: qpost_half(1, 1, state["g11"]),
            }
            # give the PE second-half proj work while the h0 posts run on
            # ACT/DVE (safe: emitted after the posts, so deps point backward)
            drain(8)
            # units drained per attention iteration: all 48 second-half proj
            # units must be emitted before their boundary posts
            drain_n = {0: 3, 1: 2, 2: 1, 3: 1}

            # ---- attention + output, per query block ---------------------
            for cb in range(NQB):
                q0 = QB * cb
                jmax = 4 * cb + 3
                for g in range(2):
                    ot = [smallps.tile([65, QB], f32, tag="small",
                                       name=f"ot{cb}{g}{h}") for h in range(2)]

                    def emit_pv(pv):
                        pj_, pex, pp0 = pv
                        for h in range(2):
                            mm = nc.tensor.matmul(
                                ot[h][:, pp0:QB], vr[pj_],
                                pex[:, QB * h + pp0:QB * h + QB],
                                start=(pj_ == 0), stop=(pj_ == jmax))
                            if h == 1:
                                mm.ins.ldweights = False

                    pend = []
                    for j in range(jmax + 1):
                        p0 = max(128 * j - q0, 0)
                        diag = 128 * j >= q0
                        sc = bigps.tile([128, 2 * QB], f32, tag="big",
                                        name="sc")
                        for h in range(2):
                            r0 = 64 * h if ROWTILE else 0
                            qsrc = (qro[g][r0:r0 + 64, q0 + p0:q0 + QB]
                                    if ROWTILE else
                                    qro4[2 * g + h][:, q0 + p0:q0 + QB])
                            nc.tensor.matmul(
                                sc[:, QB * h + p0:QB * h + QB],
                                kro2[r0:r0 + 64, 128 * j:128 * j + 128],
                                qsrc,
                                start=True, stop=True)
                        ex = epool.tile([128, 2 * QB], f16, tag="ex")
                        if p0 == 0:
                            nc.scalar.activation(out=ex, in_=sc, func=EXP,
                                                 scale=rk2[:, j:j + 1])
                        else:
                            for h in range(2):
                                nc.scalar.activation(
                                    out=ex[:, QB * h + p0:QB * h + QB],
                                    in_=sc[:, QB * h + p0:QB * h + QB],
                                    func=EXP, scale=rk2[:, j:j + 1])
                        if diag:
                            for h in range(2):
                                o = QB * h + p0
                                nc.vector.tensor_mul(
                                    ex[:, o:o + 128], ex[:, o:o + 128],
                                    tri_sb)
                        pend.append((j, ex, p0))
                        if len(pend) > 2:
                            emit_pv(pend.pop(0))
                        drain(drain_n[cb])
                    for pv in pend:
                        emit_pv(pv)
                        drain(1)
                    # normalize: aot = ot * (1/den), den = ones-row 64
                    for h in range(2):
                        den = work.tile([1, QB], f32, tag="den", bufs=2)
                        nc.scalar.activation(out=den, in_=ot[h][64:65, :],
                                             func=CPY, scale=1.0, bias=0.0)
                        rden = work.tile([1, QB], f32, tag="rden", bufs=2)
                        nc.vector.reciprocal_approx_fast(out=rden, in_=den)
                        rdenb = work.tile([1, QB], f16, tag="rdenb", bufs=2)
                        nc.vector.tensor_copy(out=rdenb, in_=rden)
                        bs = work.tile([64, QB], f16, tag="bs", bufs=2)
                        nc.gpsimd.partition_broadcast(bs, rdenb, channels=64)
                        nc.vector.tensor_mul(
                            aot[g][64 * h:64 * h + 64, q0:q0 + QB],
                            ot[h][0:64, :], bs)
                    post = boundary_posts.get((cb, g))
                    if post is not None:
                        post()
                    if (cb, g) == (1, 0):
                        # pjh slot is free once qpost(1,1) is emitted; wo
                        # units (which reuse it) may enter the queue now
                        for m in range(0, 4):
                            for hdp in range(2):
                                fillers.append(mk_wo_unit(0, m, hdp))
                    drain(2)
                if cb >= 1:
                    for m in range(4 * cb, 4 * cb + 4):
                        for hdp in range(2):
                            fillers.append(mk_wo_unit(cb, m, hdp))
            drain(len(fillers))
    nc.compile()
    return nc


LDW_OPT = True


def _enable_ldw_opt():
    """Ask walrus to optimize LDWEIGHTS scheduling for this kernel's NEFF."""
    from concourse import bass_utils as _bu
    if getattr(_bu, "_ldw_opt_wrapped", False):
        return
    _orig = _bu.run_command

    def _rc(cmd, *a, **kw):
        cmd = ["--enable-ldw-opt=true" if c == "--enable-ldw-opt=false" else c
               for c in cmd]
        return _orig(cmd, *a, **kw)

    _bu.run_command = _rc
    _bu._ldw_opt_wrapped = True


def _get_nc():
    if "nc" not in _CACHE:
        _ensure_ntff_hook()
        if LDW_OPT:
            _enable_ldw_opt()
        _CACHE["nc"] = _build_nc()
    return _CACHE["nc"]


def _make_tables(qn_w, kn_w, start_pos):
    inv = THETA ** (-np.arange(0, HD, 2, dtype=np.float64) / HD)  # (32,)
    pos = float(start_pos) + np.arange(SEQ, dtype=np.float64)
    ang = inv[:, None] * pos[None, :]  # (32, SEQ)
    c, s = np.cos(ang), np.sin(ang)

    def tabs(gain):
        g = gain.astype(np.float64)
        cosg = np.concatenate([g[0:32, None] * c, g[32:64, None] * c], axis=0)
        sing = np.concatenate([-g[32:64, None] * s, g[0:32, None] * s], axis=0)
        return cosg.astype(ml_dtypes.bfloat16), sing.astype(ml_dtypes.bfloat16)

    cq, sq_ = tabs(np.asarray(qn_w))
    ck, sk = tabs(np.asarray(kn_w))
    return (np.ascontiguousarray(np.tile(cq, (2, 1))),
            np.ascontiguousarray(np.tile(sq_, (2, 1))), ck, sk)


def _prep_in_maps(x, Wq, Wk, Wv, Wo, qn_w, kn_w, start_pos):
    xT = np.asarray(x)[0].T.astype(ml_dtypes.bfloat16)
    xt = np.ascontiguousarray(xT.reshape(16, 128, SEQ).transpose(1, 0, 2))
    cosq, sinq, cosk, sink = _make_tables(qn_w, kn_w, start_pos)
    tri = np.triu(np.ones((128, 128))).astype(ml_dtypes.bfloat16)
    idn = np.eye(128, dtype=ml_dtypes.bfloat16)
    ob = np.zeros((128, 65), ml_dtypes.bfloat16)
    ob[0:64, 0] = 1.0
    ob[64:128, 64] = 1.0
    Wq, Wk, Wv, Wo = (np.asarray(a) for a in (Wq, Wk, Wv, Wo))
    in_maps = []
    for c in range(NCORES):
        wq_c = np.ascontiguousarray(
            Wq[:, 256 * c:256 * (c + 1)].astype(ml_dtypes.bfloat16)
            .reshape(16, 128, 256).transpose(1, 0, 2))
        wkv_c = np.ascontiguousarray(np.concatenate(
            [Wk[:, HD * c:HD * (c + 1)], Wv[:, HD * c:HD * (c + 1)]],
            axis=1).astype(ml_dtypes.bfloat16)
            .reshape(16, 128, 128).transpose(1, 0, 2))
        wo_c = np.ascontiguousarray(
            Wo[256 * c:256 * (c + 1), :].astype(ml_dtypes.bfloat16)
            .reshape(2, 128, DIM).transpose(1, 0, 2))
        in_maps.append({"xt": xt, "wq": wq_c, "wkv": wkv_c, "wo": wo_c,
                        "cosq": cosq, "sinq": sinq, "cosk": cosk, "sink": sink,
                        "tri": tri, "idn": idn, "ob": ob})
    return in_maps


def run(inputs, trace=False, **kw):
    from concourse import bass_utils
    nc = _get_nc()
    in_maps = _prep_in_maps(
        inputs["x"], inputs["Wq"], inputs["Wk"], inputs["Wv"], inputs["Wo"],
        inputs["qn_w"], inputs["kn_w"], inputs["start_pos"])
    res = bass_utils.run_bass_kernel_spmd(
        nc, in_maps, core_ids=list(range(NCORES)), trace=trace, **kw)
    y = np.zeros((SEQ, DIM), np.float32)
    for r in res.results:
        y += r["y"].reshape(SEQ, DIM).astype(np.float32)
    return y.reshape(1, SEQ, DIM), res


def kernel(x, Wq, Wk, Wv, Wo, qn_w, kn_w, mask, start_pos):
    out, _ = run(dict(x=x, Wq=Wq, Wk=Wk, Wv=Wv, Wo=Wo, qn_w=qn_w, kn_w=kn_w,
                      mask=mask, start_pos=start_pos))
    return out


# revision 49
# speedup vs baseline: 1.1443x; 1.1443x over previous
"""GQA kernel for trn2, 8 NeuronCores, tensor-parallel over heads.

Sharding: 4 q heads + 1 kv head per core (column-split Wq/Wk/Wv, row-split Wo),
partial outputs summed on host. All matmul operands bf16, fp32 PSUM accumulation.

v2 layout/schedule (vs baseline):
  - weights DMA'd before x; kv/q projections accumulate per arriving x chunk
    (halved over SEQ so 3 group-halves fit the PSUM budget)
  - scores row-tiled on the PE: head pair (2g, 2g+1) at tile_position (0,0)/
    (64,0) run concurrently (K=64 each) into one [128, 1024] psum tile
  - causal mask folded into scores via maskT.T @ I accumulation on diagonal
    128-chunks (no DVE tri-mul)
  - one exp per (pair, qblock, kchunk) with per-partition k-RMS scale folded;
    ACT stays in the exp table set during attention
  - softmax denominator via ones-row 64 of the PV lhsT; 1/den via DVE
    reciprocal_approx_fast; column-broadcast via gpsimd; normalize on DVE
  - Wo of block cb and second-half projections emitted as PE filler inside
    the next block's exp-wait gaps
"""

import sys
import types
import numpy as np
import ml_dtypes

for _p in ("/opt/trn_rl_repo",):
    if _p not in sys.path:
        sys.path.append(_p)

SEQ = 2048
DIM = 2048
HD = 64
NCORES = 8
EPS = 1e-6
THETA = 10000.0
QB = 512          # attention query block (1 psum bank per head)
NQB = SEQ // QB   # 4
HB = 1024         # projection column half
MASKV = -30000.0
ROWTILE = True    # run the head pair's score matmuls on PE row strips 0/1

_CACHE = {}


def _ensure_ntff_hook():
    """Re-register the NTFF profile hook the boot drops (stub antenv)."""
    if "antenv.axon_hooks" in sys.modules:
        return
    try:
        import antenv
        m = types.ModuleType("antenv.axon_hooks")
        hook = [None]
        m.set_axon_ntff_profile_hook = lambda h: hook.__setitem__(0, h)
        m.get_axon_ntff_profile_hook = lambda: hook[0]
        sys.modules["antenv.axon_hooks"] = m
        antenv.axon_hooks = m
        from trn_agent_boot.trn_boot import _ntff_profile_via_ctypes
        m.set_axon_ntff_profile_hook(
            _ntff_profile_via_ctypes("/opt/axon/libaxon_pjrt.so"))
    except Exception:
        pass


def _build_nc():
    import concourse.mybir as mybir
    import concourse.tile as tile
    from concourse import bacc

    f32, f16 = mybir.dt.float32, mybir.dt.bfloat16
    EXP = mybir.ActivationFunctionType.Exp
    RSQ = mybir.ActivationFunctionType.Abs_reciprocal_sqrt
    SQR = mybir.ActivationFunctionType.Square
    CPY = mybir.ActivationFunctionType.Copy

    nc = bacc.Bacc("TRN2", target_bir_lowering=False, debug=False,
                   num_devices=NCORES)

    d_xt = nc.dram_tensor("xt", (128, 16, SEQ), f16, kind="ExternalInput")
    d_wq = nc.dram_tensor("wq", (128, 16, 256), f16, kind="ExternalInput")
    d_wkv = nc.dram_tensor("wkv", (128, 16, 128), f16, kind="ExternalInput")
    d_wo = nc.dram_tensor("wo", (128, 2, DIM), f16, kind="ExternalInput")
    d_cosq = nc.dram_tensor("cosq", (128, SEQ), f16, kind="ExternalInput")
    d_sinq = nc.dram_tensor("sinq", (128, SEQ), f16, kind="ExternalInput")
    d_cosk = nc.dram_tensor("cosk", (64, SEQ), f16, kind="ExternalInput")
    d_sink = nc.dram_tensor("sink", (64, SEQ), f16, kind="ExternalInput")
    d_tri = nc.dram_tensor("tri", (128, 128), f16, kind="ExternalInput")
    d_idn = nc.dram_tensor("idn", (128, 128), f16, kind="ExternalInput")
    d_ob = nc.dram_tensor("ob", (128, 65), f16, kind="ExternalInput")
    d_y = nc.dram_tensor("y", (16, 128, DIM), f16, kind="ExternalOutput")

    with tile.TileContext(nc) as tc:
        from contextlib import ExitStack
        with ExitStack() as ctx:
            kconst = ctx.enter_context(tc.tile_pool(name="kconst", bufs=1))
            xpool = ctx.enter_context(tc.tile_pool(name="xp", bufs=1))
            work = ctx.enter_context(tc.tile_pool(name="work", bufs=1))
            vpool = ctx.enter_context(tc.tile_pool(name="vp", bufs=1))
            epool = ctx.enter_context(tc.tile_pool(name="ep", bufs=4))
            yspool = ctx.enter_context(tc.tile_pool(name="ysp", bufs=2))
            bigps = ctx.enter_context(
                tc.tile_pool(name="bigps", bufs=2, space="PSUM"))
            smallps = ctx.enter_context(
                tc.tile_pool(name="smallps", bufs=2, space="PSUM"))

            # ---- DMAs: proj weights, then x (chunk groups), then consts
            wq_sb = kconst.tile([128, 16, 256], f16, tag="wq")
            nc.sync.dma_start(out=wq_sb, in_=d_wq[:, :, :])
            wkv_sb = kconst.tile([128, 16, 128], f16, tag="wkv")
            nc.sync.dma_start(out=wkv_sb, in_=d_wkv[:, :, :])
            XG = [(0, 2), (2, 4), (4, 8), (8, 12), (12, 16)]
            xtg = []
            for gi, (a, b) in enumerate(XG):
                t = xpool.tile([128, b - a, SEQ], f16, tag=f"xtg{gi}",
                               name=f"xtg{gi}")
                nc.sync.dma_start(out=t, in_=d_xt[:, a:b, :])
                xtg.append(t)

            def xts(k):
                for gi, (a, b) in enumerate(XG):
                    if a <= k < b:
                        return xtg[gi][:, k - a, :]

            wo_sb = kconst.tile([128, 2, DIM], f16, tag="wo")
            nc.sync.dma_start(out=wo_sb, in_=d_wo[:, :, :])
            cosq_sb = kconst.tile([128, SEQ], f16, tag="cosq")
            nc.sync.dma_start(out=cosq_sb, in_=d_cosq[:, :])
            sinq_sb = kconst.tile([128, SEQ], f16, tag="sinq")
            nc.sync.dma_start(out=sinq_sb, in_=d_sinq[:, :])
            cosk_sb = kconst.tile([64, SEQ], f16, tag="cosk")
            nc.sync.dma_start(out=cosk_sb, in_=d_cosk[:, :])
            sink_sb = kconst.tile([64, SEQ], f16, tag="sink")
            nc.sync.dma_start(out=sink_sb, in_=d_sink[:, :])
            tri_sb = kconst.tile([128, 128], f16, tag="tri")
            nc.sync.dma_start(out=tri_sb, in_=d_tri[:, :])
            idn_sb = kconst.tile([128, 128], f16, tag="idn")
            nc.sync.dma_start(out=idn_sb, in_=d_idn[:, :])
            ob_sb = kconst.tile([128, 65], f16, tag="ob")
            nc.sync.dma_start(out=ob_sb, in_=d_ob[:, :])
            eps_sb = kconst.tile([128, 1], f32, tag="eps")
            nc.vector.memset(eps_sb, EPS)
            eps64_sb = kconst.tile([128, 1], f32, tag="eps64")
            nc.vector.memset(eps64_sb, float(HD) * EPS)

            # persistent projection-phase results
            if ROWTILE:
                qro = [kconst.tile([128, SEQ], f16, tag=f"qro{g}",
                                   name=f"qro{g}") for g in range(2)]
            else:
                qro4 = [kconst.tile([64, SEQ], f16, tag=f"qro4_{i}",
                                    name=f"qro4_{i}") for i in range(4)]
            kro2 = kconst.tile([128, SEQ], f16, tag="kro2")  # kro duplicated
            rk2 = kconst.tile([128, 16], f32, tag="rk2")     # exp scale/kchunk
            rsq_ = [[kconst.tile([1, SEQ], f16, tag=f"rs{g}{r}",
                                 name=f"rs{g}{r}") for r in range(2)]
                    for g in range(2)]           # q-norm rsqrt rows
            vr = [vpool.tile([128, 65], f16, tag=f"vr{j}", name=f"vr{j}")
                  for j in range(16)]            # [v.T | 1] per 128-k-chunk
            aot = [kconst.tile([128, SEQ], f16, tag=f"aot{g}", name=f"aot{g}")
                   for g in range(2)]            # normalized attn out (Wo lhsT)

            # ---- projection helpers (per column half) --------------------
            def qpost_half(g, half, pj):
                """rope + q-norm for head pair g, seq cols half*HB..+HB."""
                h0 = HB * half
                q16 = work.tile([128, HB], f16, tag="q16", bufs=3)
                nc.scalar.activation(out=q16, in_=pj, func=CPY,
                                     scale=1.0, bias=0.0)
                sq = work.tile([128, HB], f16, tag="sq", bufs=3)
                nc.scalar.activation(out=sq, in_=pj, func=SQR,
                                     scale=1.0, bias=0.0)
                for n in range(2):
                    sp = smallps.tile([65, 512], f32, tag="small")
                    nc.tensor.matmul(sp, ob_sb,
                                     sq[:, 512 * n:512 * n + 512],
                                     start=True, stop=True)
                    for r in range(2):
                        nc.scalar.activation(
                            out=rsq_[g][r][:, h0 + 512 * n:h0 + 512 * n + 512],
                            in_=sp[64 * r:64 * r + 1, :], func=RSQ,
                            scale=1.0 / HD, bias=eps_sb[0:1, :])
                rot = work.tile([128, HB], f16, tag="rot", bufs=2)
                for (o, s) in ((0, 32), (32, 0), (64, 96), (96, 64)):
                    nc.vector.tensor_copy(out=rot[o:o + 32, :],
                                          in_=q16[s:s + 32, :])
                for r in range(2):
                    p = 64 * r
                    bq = work.tile([64, HB], f16, tag="bq", bufs=2)
                    nc.gpsimd.partition_broadcast(
                        bq, rsq_[g][r][:, h0:h0 + HB], channels=64)
                    tm = work.tile([64, HB], f16, tag="tm", bufs=2)
                    nc.vector.tensor_mul(tm, rot[p:p + 64, :],
                                         sinq_sb[p:p + 64, h0:h0 + HB])
                    hh = work.tile([64, HB], f16, tag="hh", bufs=2)
                    nc.vector.tensor_mul(hh, q16[p:p + 64, :],
                                         cosq_sb[p:p + 64, h0:h0 + HB])
                    nc.vector.tensor_add(hh, hh, tm)
                    qdst = (qro[g][p:p + 64, h0:h0 + HB] if ROWTILE
                            else qro4[2 * g + r][:, h0:h0 + HB])
                    nc.vector.tensor_mul(qdst, hh, bq)

            def kvpost_half(half, pj):
                h0 = HB * half
                k16 = work.tile([64, HB], f16, tag="q16", bufs=3)
                nc.scalar.activation(out=k16, in_=pj[0:64, :], func=CPY,
                                     scale=1.0, bias=0.0)
                sqk = work.tile([64, HB], f16, tag="sq", bufs=3)
                nc.scalar.activation(out=sqk, in_=pj[0:64, :], func=SQR,
                                     scale=1.0, bias=0.0)
                v16 = work.tile([64, HB], f16, tag="v16", bufs=2)
                nc.scalar.activation(out=v16, in_=pj[64:128, :], func=CPY,
                                     scale=1.0, bias=0.0)
                pc = smallps.tile([128, 8], f32, tag="small")
                for j in range(8):
                    nc.tensor.matmul(pc[:, j:j + 1],
                                     sqk[:, 128 * j:128 * j + 128],
                                     ob_sb[0:64, 0:1],
                                     start=True, stop=True)
                nc.scalar.activation(out=rk2[:, 8 * half:8 * half + 8],
                                     in_=pc, func=RSQ,
                                     scale=1.0, bias=eps64_sb)
                rotk = work.tile([64, HB], f16, tag="rot", bufs=2)
                for (o, s) in ((0, 32), (32, 0)):
                    nc.vector.tensor_copy(out=rotk[o:o + 32, :],
                                          in_=k16[s:s + 32, :])
                tmk = work.tile([64, HB], f16, tag="tm", bufs=2)
                nc.vector.tensor_mul(tmk, rotk, sink_sb[:, h0:h0 + HB])
                hk = work.tile([64, HB], f16, tag="hh", bufs=2)
                nc.vector.tensor_mul(hk, k16, cosk_sb[:, h0:h0 + HB])
                nc.vector.tensor_add(kro2[0:64, h0:h0 + HB], hk, tmk)
                nc.vector.tensor_copy(out=kro2[64:128, h0:h0 + HB],
                                      in_=kro2[0:64, h0:h0 + HB])
                for j in range(8):
                    tp = smallps.tile([128, 64], f16, tag="small")
                    nc.tensor.transpose(tp, v16[:, 128 * j:128 * j + 128],
                                        idn_sb[0:64, 0:64])
                    jj = 8 * half + j
                    nc.vector.tensor_copy(out=vr[jj][:, 0:64], in_=tp)
                    nc.vector.memset(vr[jj][:, 64:65], 1.0)

            # ---- first halves: interleaved per x chunk (DMA-paced) -------
            pj_kv0 = bigps.tile([128, HB], f32, tag="big", name="pjkv0")
            pj_g00 = bigps.tile([128, HB], f32, tag="big", name="pjg00")
            pj_g10 = bigps.tile([128, HB], f32, tag="pjh", bufs=1,
                                name="pjg10")
            for k in range(16):
                for pj, wsl in ((pj_kv0, wkv_sb[:, k, :]),
                                (pj_g00, wq_sb[:, k, 0:128]),
                                (pj_g10, wq_sb[:, k, 128:256])):
                    for n in range(2):
                        mm = nc.tensor.matmul(
                            pj[:, 512 * n:512 * n + 512], wsl,
                            xts(k)[:, 512 * n:512 * n + 512],
                            start=(k == 0), stop=(k == 15))
                        if n == 1:
                            mm.ins.ldweights = False
            kvpost_half(0, pj_kv0)
            qpost_half(0, 0, pj_g00)
            qpost_half(1, 0, pj_g10)

            # ---- second halves queued as PE filler work ------------------
            fillers = []
            state = {}

            def mk_proj_unit(key, wsl, k, name):
                def unit():
                    if k == 0:
                        state[key] = bigps.tile([128, HB], f32, tag="pjh",
                                                bufs=1, name=name)
                    pj = state[key]
                    for n in range(2):
                        c = HB + 512 * n
                        mm = nc.tensor.matmul(pj[:, 512 * n:512 * n + 512],
                                              wsl(k), xts(k)[:, c:c + 512],
                                              start=(k == 0), stop=(k == 15))
                        if n == 1:
                            mm.ins.ldweights = False
                return unit

            for k in range(16):
                fillers.append(mk_proj_unit(
                    "kv1", lambda k: wkv_sb[:, k, :], k, "pjkv1"))
            for k in range(16):
                fillers.append(mk_proj_unit(
                    "g01", lambda k: wq_sb[:, k, 0:128], k, "pjg01"))
            for k in range(16):
                fillers.append(mk_proj_unit(
                    "g11", lambda k: wq_sb[:, k, 128:256], k, "pjg11"))

            def drain(n):
                for _ in range(min(n, len(fillers))):
                    fillers.pop(0)()

            def mk_wo_unit(cb, m, hdp):
                def unit():
                    if hdp == 0:
                        state[f"ys{m}"] = yspool.tile([128, DIM], f16,
                                                      tag="ysm",
                                                      name=f"ysm{m}")
                    ysm = state[f"ys{m}"]
                    c0, c1 = 1024 * hdp, 1024 * hdp + 512
                    if cb == NQB - 1:   # tail: double-buffer via the sc slots
                        yp = bigps.tile([128, 1024], f32, tag="big",
                                        name="yp")
                    else:
                        yp = bigps.tile([128, 1024], f32, tag="pjh", bufs=1,
                                        name="yp")
                    for g in range(2):
                        nc.tensor.matmul(yp[:, 0:512],
                                         aot[g][:, 128 * m:128 * m + 128],
                                         wo_sb[:, g, c0:c0 + 512],
                                         start=(g == 0), stop=(g == 1))
                        mm = nc.tensor.matmul(
                            yp[:, 512:1024], aot[g][:, 128 * m:128 * m + 128],
                            wo_sb[:, g, c1:c1 + 512],
                            start=(g == 0), stop=(g == 1))
                        mm.ins.ldweights = False
                    if cb == NQB - 1:
                        nc.scalar.activation(out=ysm[:, c0:c0 + 512],
                                             in_=yp[:, 0:512],
                                             func=CPY, scale=1.0, bias=0.0)
                    else:
                        nc.vector.tensor_copy(out=ysm[:, c0:c0 + 512],
                                              in_=yp[:, 0:512])
                    nc.vector.tensor_copy(out=ysm[:, c1:c1 + 512],
                                          in_=yp[:, 512:1024])
                    if hdp == 1:
                        nc.sync.dma_start(out=d_y[m], in_=ysm)
                return unit

            # posts for the second halves, emitted at (cb, g) boundaries
            # where the small-psum ot slots are free (deadlock avoidance)
            boundary_posts = {
                (0, 0): lambda: kvpost_half(1, state["kv1"]),
                (0, 1): lambda: qpost_half(0, 1, state["g01"]),
                (1, 0): lambda: qpost_half(1, 1, state["g11"]),
            }
            # give the PE second-half proj work while the h0 posts run on
            # ACT/DVE (safe: emitted after the posts, so deps point backward)
            drain(8)
            # units drained per attention iteration: all 48 second-half proj
            # units must be emitted before their boundary posts
            drain_n = {0: 3, 1: 2, 2: 1, 3: 1}

            # ---- attention + output, per query block ---------------------
            for cb in range(NQB):
                q0 = QB * cb
                jmax = 4 * cb + 3
                for g in range(2):
                    ot = [smallps.tile([65, QB], f32, tag="small",
                                       name=f"ot{cb}{g}{h}") for h in range(2)]

                    def emit_pv(pv):
                        pj_, pex, pp0 = pv
                        for h in range(2):
                            mm = nc.tensor.matmul(
                                ot[h][:, pp0:QB], vr[pj_],
                                pex[:, QB * h + pp0:QB * h + QB],
                                start=(pj_ == 0), stop=(pj_ == jmax))
                            if h == 1:
                                mm.ins.ldweights = False

                    pend = []
                    for j in range(jmax + 1):
                        p0 = max(128 * j - q0, 0)
                        diag = 128 * j >= q0
                        sc = bigps.tile([128, 2 * QB], f32, tag="big",
                                        name="sc")
                        for h in range(2):
                            r0 = 64 * h if ROWTILE else 0
                            qsrc = (qro[g][r0:r0 + 64, q0 + p0:q0 + QB]
                                    if ROWTILE else
                                    qro4[2 * g + h][:, q0 + p0:q0 + QB])
                            nc.tensor.matmul(
                                sc[:, QB * h + p0:QB * h + QB],
                                kro2[r0:r0 + 64, 128 * j:128 * j + 128],
                                qsrc,
                                start=True, stop=True)
                        ex = epool.tile([128, 2 * QB], f16, tag="ex")
                        if p0 == 0:
                            nc.scalar.activation(out=ex, in_=sc, func=EXP,
                                                 scale=rk2[:, j:j + 1])
                        else:
                            for h in range(2):
                                nc.scalar.activation(
                                    out=ex[:, QB * h + p0:QB * h + QB],
                                    in_=sc[:, QB * h + p0:QB * h + QB],
                                    func=EXP, scale=rk2[:, j:j + 1])
                        if diag:
                            for h in range(2):
                                o = QB * h + p0
                                nc.vector.tensor_mul(
                                    ex[:, o:o + 128], ex[:, o:o + 128],
                                    tri_sb)
                        pend.append((j, ex, p0))
                        if len(pend) > 2:
                            emit_pv(pend.pop(0))
                        drain(drain_n[cb])
                    for pv in pend:
                        emit_pv(pv)
                        drain(1)
                    # normalize: aot = ot * (1/den), den = ones-row 64
                    for h in range(2):
                        den = work.tile([1, QB], f32, tag="den", bufs=2)
                        nc.scalar.activation(out=den, in_=ot[h][64:65, :],
                                             func=CPY, scale=1.0, bias=0.0)
                        rden = work.tile([1, QB], f32, tag="rden", bufs=2)
                        nc.vector.reciprocal_approx_fast(out=rden, in_=den)
                        rdenb = work.tile([1, QB], f16, tag="rdenb", bufs=2)
                        nc.vector.tensor_copy(out=rdenb, in_=rden)
                        bs = work.tile([64, QB], f16, tag="bs", bufs=2)
                        nc.gpsimd.partition_broadcast(bs, rdenb, channels=64)
                        nc.vector.tensor_mul(
                            aot[g][64 * h:64 * h + 64, q0:q0 + QB],
                            ot[h][0:64, :], bs)
                    post = boundary_posts.get((cb, g))
                    if post is not None:
                        post()
                    if (cb, g) == (1, 0):
                        # pjh slot is free once qpost(1,1) is emitted; wo
                        # units (which reuse it) may enter the queue now
                        for m in range(0, 4):
                            for hdp in range(2):
                                fillers.append(mk_wo_unit(0, m, hdp))
                    drain(2)
                if cb >= 1:
                    for m in range(4 * cb, 4 * cb + 4):
                        for hdp in range(2):
                            fillers.append(mk_wo_unit(cb, m, hdp))
            drain(len(fillers))
    nc.compile()
    return nc


LDW_OPT = True


def _enable_ldw_opt():
    """Ask walrus to optimize LDWEIGHTS scheduling for this kernel's NEFF."""
    from concourse import bass_utils as _bu
    if getattr(_bu, "_ldw_opt_wrapped", False):
        return
    _orig = _bu.run_command

    def _rc(cmd, *a, **kw):
        cmd = ["--enable-ldw-opt=true" if c == "--enable-ldw-opt=false" else c
               for c in cmd]
        return _orig(cmd, *a, **kw)

    _bu.run_command = _rc
    _bu._ldw_opt_wrapped = True


def _get_nc():
    if "nc" not in _CACHE:
        _ensure_ntff_hook()
        if LDW_OPT:
            _enable_ldw_opt()
        _CACHE["nc"] = _build_nc()
    return _CACHE["nc"]


def _make_tables(qn_w, kn_w, start_pos):
    inv = THETA ** (-np.arange(0, HD, 2, dtype=np.float64) / HD)  # (32,)
    pos = float(start_pos) + np.arange(SEQ, dtype=np.float64)
    ang = inv[:, None] * pos[None, :]  # (32, SEQ)
    c, s = np.cos(ang), np.sin(ang)

    def tabs(gain):
        g = gain.astype(np.float64)
        cosg = np.concatenate([g[0:32, None] * c, g[32:64, None] * c], axis=0)
        sing = np.concatenate([-g[32:64, None] * s, g[0:32, None] * s], axis=0)
        return cosg.astype(ml_dtypes.bfloat16), sing.astype(ml_dtypes.bfloat16)

    cq, sq_ = tabs(np.asarray(qn_w))
    ck, sk = tabs(np.asarray(kn_w))
    return (np.ascontiguousarray(np.tile(cq, (2, 1))),
            np.ascontiguousarray(np.tile(sq_, (2, 1))), ck, sk)


def _prep_in_maps(x, Wq, Wk, Wv, Wo, qn_w, kn_w, start_pos):
    xT = np.asarray(x)[0].T.astype(ml_dtypes.bfloat16)
    xt = np.ascontiguousarray(xT.reshape(16, 128, SEQ).transpose(1, 0, 2))
    cosq, sinq, cosk, sink = _make_tables(qn_w, kn_w, start_pos)
    tri = np.triu(np.ones((128, 128))).astype(ml_dtypes.bfloat16)
    idn = np.eye(128, dtype=ml_dtypes.bfloat16)
    ob = np.zeros((128, 65), ml_dtypes.bfloat16)
    ob[0:64, 0] = 1.0
    ob[64:128, 64] = 1.0
    Wq, Wk, Wv, Wo = (np.asarray(a) for a in (Wq, Wk, Wv, Wo))
    in_maps = []
    for c in range(NCORES):
        wq_c = np.ascontiguousarray(
            Wq[:, 256 * c:256 * (c + 1)].astype(ml_dtypes.bfloat16)
            .reshape(16, 128, 256).transpose(1, 0, 2))
        wkv_c = np.ascontiguousarray(np.concatenate(
            [Wk[:, HD * c:HD * (c + 1)], Wv[:, HD * c:HD * (c + 1)]],
            axis=1).astype(ml_dtypes.bfloat16)
            .reshape(16, 128, 128).transpose(1, 0, 2))
        wo_c = np.ascontiguousarray(
            Wo[256 * c:256 * (c + 1), :].astype(ml_dtypes.bfloat16)
            .reshape(2, 128, DIM).transpose(1, 0, 2))
        in_maps.append({"xt": xt, "wq": wq_c, "wkv": wkv_c, "wo": wo_c,
                        "cosq": cosq, "sinq": sinq, "cosk": cosk, "sink": sink,
                        "tri": tri, "idn": idn, "ob": ob})
    return in_maps


def run(inputs, trace=False, **kw):
    from concourse import bass_utils
    nc = _get_nc()
    in_maps = _prep_in_maps(
        inputs["x"], inputs["Wq"], inputs["Wk"], inputs["Wv"], inputs["Wo"],
        inputs["qn_w"], inputs["kn_w"], inputs["start_pos"])
    res = bass_utils.run_bass_kernel_spmd(
        nc, in_maps, core_ids=list(range(NCORES)), trace=trace, **kw)
    y = np.zeros((SEQ, DIM), np.float32)
    for r in res.results:
        y += r["y"].reshape(SEQ, DIM).astype(np.float32)
    return y.reshape(1, SEQ, DIM), res


def kernel(x, Wq, Wk, Wv, Wo, qn_w, kn_w, mask, start_pos):
    out, _ = run(dict(x=x, Wq=Wq, Wk=Wk, Wv=Wv, Wo=Wo, qn_w=qn_w, kn_w=kn_w,
                      mask=mask, start_pos=start_pos))
    return out
